# revision 11
# baseline (speedup 1.0000x reference)
"""Involution-bin block on 8 TRN2 NeuronCores, batch-parallel (1 sample/core).

Per-core Bass program (compute in bf16, accumulation f32):
  sign -> conv1x1 (TensorE, block-diag weights over (c,half) packing) -> BN1
  (per-sample stats; cross-half combine via a tiny matmul) -> PReLU (DVE
  max-trick) -> involution: kernel branch folded to one matmul (span@reduce
  pre-multiplied on host), unfold*ker computed in a pixel-transposed layout
  (xbar DMA transpose + restride) as 9 DVE broadcast-multiplies + 9
  shifted-identity TensorE matmuls accumulating in PSUM -> BN2+ReLU
  (ScalarE) -> conv1x1 -> BN3 -> + x residual.

BatchNorm uses per-sample statistics (batch-exact stats would need
cross-core collectives; measured end-to-end L2 error ~1.3e-2 < 2e-2 gate).
pre/post conv biases cancel exactly through the following BN; prelu after
relu is the identity; mid bias is folded in only if nonzero.
"""

import sys

sys.path.insert(0, "/opt/trn_rl_repo")

import numpy as np
import ml_dtypes

import concourse.bass as bass
import concourse.bacc as bacc
import concourse.mybir as mybir
from concourse.tile import TileContext

BF = mybir.dt.bfloat16
F32 = mybir.dt.float32

C = 64          # channels
NH = 2          # halves of the image rows
CH = C * NH     # 128 = packed partition count (ch = h*64 + c)
IL = 64         # image rows per half
W = 128         # image width (= partitions in the transposed layout)
F = IL * W      # free size per partition, c-major
NPIX = NH * F   # pixels per image
EPS = 1e-5
N_CORES = 8
NPIX_G = NPIX * N_CORES  # batch-global pixel count for BN stats
BS = 16         # il block size for the tap loop
NBLK = IL // BS
CHUNK = 2048    # c-major free-dim chunk (4 chunks of (il=16, j=128))
NCHUNK = F // CHUNK

_CACHE = {}


def _bin_w(w):
    w = np.asarray(w, np.float64)
    sf = np.mean(np.abs(w), axis=(1, 2, 3), keepdims=True)
    return (sf * np.sign(w))[:, :, 0, 0]  # (O, I)


def _bdiag(m):
    """lhsT for conv: out((o,h), q) = sum_(i,h') lhsT[(i,h'),(o,h)] rhs[(i,h'), q]."""
    o, i = m.shape
    t = np.zeros((CH, CH), np.float64)
    for h in range(NH):
        t[h * C:h * C + i, h * C:h * C + o] = m.T
    return t


def _build_nc(with_mid_bias):
    nc = bacc.Bacc()
    x_ext = nc.dram_tensor("x", [C, NH * IL, W], F32, kind="ExternalInput")
    w1_ext = nc.dram_tensor("w1bd", [CH, CH], BF, kind="ExternalInput")
    km_ext = nc.dram_tensor("km32", [CH, 32], BF, kind="ExternalInput")
    w3_ext = nc.dram_tensor("w3bd", [CH, CH], BF, kind="ExternalInput")
    i0_ext = nc.dram_tensor("imat0", [W, W], BF, kind="ExternalInput")
    ip_ext = nc.dram_tensor("imatp", [W, W], BF, kind="ExternalInput")
    im_ext = nc.dram_tensor("imatm", [W, W], BF, kind="ExternalInput")
    e_ext = nc.dram_tensor("emat", [CH, CH], F32, kind="ExternalInput")
    v_ext = nc.dram_tensor("vecs", [CH, 8], F32, kind="ExternalInput")
    y_ext = nc.dram_tensor("y", [C, NH * IL, W], F32, kind="ExternalOutput")
    cc_bufs = []
    for i in range(3):
        ci = nc.dram_tensor(f"ccin{i}", [CH, 2], F32)
        co = nc.dram_tensor(f"ccout{i}", [CH, 2], F32, addr_space="Shared")
        cc_bufs.append((ci, co))

    AT = mybir.ActivationFunctionType
    OP = mybir.AluOpType

    with TileContext(nc) as tc:
        with tc.tile_pool(name="wp", bufs=1) as wp, \
             tc.tile_pool(name="big", bufs=1) as bp, \
             tc.tile_pool(name="qp", bufs=3) as qp, \
             tc.tile_pool(name="ps", bufs=2, space="PSUM") as ps:
            # ---- weights / consts
            w1 = wp.tile([CH, CH], BF, tag="w1")
            km = wp.tile([CH, 32], BF, tag="km")
            w3 = wp.tile([CH, CH], BF, tag="w3")
            i0 = wp.tile([W, W], BF, tag="i0")
            ipm = wp.tile([W, W], BF, tag="ip")
            imm = wp.tile([W, W], BF, tag="im")
            em = wp.tile([CH, CH], F32, tag="em")
            vec = wp.tile([CH, 8], F32, tag="vec")
            for dst, src in ((w1, w1_ext), (km, km_ext), (w3, w3_ext),
                             (i0, i0_ext), (ipm, ip_ext), (imm, im_ext),
                             (em, e_ext), (vec, v_ext)):
                nc.sync.dma_start(out=dst[:], in_=src[:])

            g1, b1, a1 = vec[:, 0:1], vec[:, 1:2], vec[:, 2:3]
            g2, b2 = vec[:, 3:4], vec[:, 4:5]
            g3, b3 = vec[:, 5:6], vec[:, 6:7]
            bmid = vec[:, 7:8]

            # ---- big persistent tiles (with manual reuse)
            x_sb = bp.tile([CH, IL, W], F32, tag="x")      # also the final y
            h0 = bp.tile([CH, F], BF, tag="h0")            # sign(x); later o_cm
            u = bp.tile([CH, F], BF, tag="u")              # conv1 out; later pin
            hm = bp.tile([CH, F], BF, tag="hm")            # h_mid; later z
            t1 = bp.tile([W, IL, CH], BF, tag="t1")        # xbar out; also scratch
            hT2 = bp.tile([W, CH, IL], BF, tag="hT2")
            kcm = bp.tile([32, IL, W], BF, tag="kcm")
            kcp = bp.tile([32, IL, W], BF, tag="kcp")
            kcmm = bp.tile([32, IL, W], BF, tag="kcmm")
            kt1 = bp.tile([W, IL, 32], BF, tag="kt1")
            kt = bp.tile([W, 32, IL], BF, tag="kt")
            ktp = bp.tile([W, 32, IL], BF, tag="ktp")
            ktm = bp.tile([W, 32, IL], BF, tag="ktm")
            outT = bp.tile([W, IL, CH], BF, tag="outT")    # later bf16 scratch
            st = bp.tile([CH, 16], F32, tag="st")          # stats staging
            sv = bp.tile([CH, 12], F32, tag="sv")          # affine results

            def cslice(t, k):
                return t[:, k * CHUNK:(k + 1) * CHUNK]

            scr = t1[:].rearrange("a b c -> a (b c)")

            # ---- load x, sign
            ILC = IL // NCHUNK
            for k in range(NCHUNK):
                nc.sync.dma_start(
                    out=x_sb[:, k * ILC:(k + 1) * ILC, :],
                    in_=x_ext[:].rearrange("c (h il) w -> h c il w",
                                           h=NH)[:, :, k * ILC:(k + 1) * ILC, :])
            xf = x_sb[:].rearrange("p il w -> p (il w)")
            for k in range(NCHUNK):
                nc.scalar.activation(cslice(h0, k), cslice(xf, k), AT.Sign)

            # ---- conv1 (512-col matmuls), evict + BN1 partial stats
            for k in range(NCHUNK):
                pt = ps.tile([CH, CHUNK], F32, tag="mm")
                for m in range(CHUNK // 512):
                    nc.tensor.matmul(pt[:, m * 512:(m + 1) * 512], w1[:],
                                     cslice(h0, k)[:, m * 512:(m + 1) * 512],
                                     start=True, stop=True)
                nc.scalar.activation(cslice(u, k), pt[:], AT.Copy,
                                     accum_out=st[:, k:k + 1])
            for k in range(NCHUNK):
                nc.vector.scalar_tensor_tensor(
                    cslice(scr, k), cslice(u, k), 1.0, cslice(u, k),
                    OP.mult, OP.mult, accum_out=st[:, 4 + k:5 + k])

            def bn_affine(gamma, beta, scol, cc):
                """s,t from st[:,0:4] (sums) and st[:,4:8] (sumsqs) -> sv.
                Partial sums are all-reduced across the 8 cores (batch BN)."""
                s_, t_ = sv[:, scol:scol + 1], sv[:, scol + 1:scol + 2]
                m2 = sv[:, scol + 2:scol + 3]
                r2 = sv[:, scol + 3:scol + 4]
                nc.vector.tensor_reduce(st[:, 12:13], st[:, 0:4],
                                        mybir.AxisListType.X, OP.add)
                nc.vector.tensor_reduce(st[:, 13:14], st[:, 4:8],
                                        mybir.AxisListType.X, OP.add)
                ci, co = cc
                nc.sync.dma_start(out=ci[:], in_=st[:, 12:14])
                nc.gpsimd.collective_compute(
                    "AllReduce", OP.add, ins=[ci[:]], outs=[co[:]],
                    replica_groups=[list(range(N_CORES))])
                nc.sync.dma_start(out=st[:, 12:14], in_=co[:])
                pe = ps.tile([CH, 2], F32, tag="mm")
                nc.tensor.matmul(pe[:], em[:], st[:, 12:14], start=True, stop=True)
                mean, msq = st[:, 14:15], st[:, 15:16]
                nc.vector.tensor_scalar(mean, pe[:, 0:1], 1.0 / NPIX_G, None, OP.mult)
                nc.vector.tensor_scalar(msq, pe[:, 1:2], 1.0 / NPIX_G, None, OP.mult)
                nc.vector.scalar_tensor_tensor(m2, mean, 1.0, mean, OP.mult, OP.mult)
                nc.vector.scalar_tensor_tensor(r2, m2, -1.0, msq, OP.mult, OP.add)
                nc.vector.tensor_scalar(r2, r2, EPS, None, OP.add)
                nc.scalar.activation(m2, r2, AT.Sqrt)
                nc.vector.reciprocal(r2, m2)
                nc.vector.tensor_tensor(s_, gamma, r2, OP.mult)
                nc.vector.scalar_tensor_tensor(t_, s_, 1.0, mean, OP.mult, OP.mult)
                nc.vector.scalar_tensor_tensor(t_, t_, -1.0, beta, OP.mult, OP.add)
                return s_, t_

            s1, t1v = bn_affine(g1, b1, 0, cc_bufs[0])

            # ---- BN1 apply (ScalarE) + PReLU (DVE: max(a*y, y)) -> hm
            for k in range(NCHUNK):
                nc.scalar.activation(cslice(u, k), cslice(u, k), AT.Identity,
                                     bias=t1v, scale=s1)
            for k in range(NCHUNK):
                nc.vector.scalar_tensor_tensor(
                    cslice(hm, k), cslice(u, k), a1, cslice(u, k),
                    OP.mult, OP.max)
            if with_mid_bias:
                for k in range(NCHUNK):
                    nc.vector.tensor_scalar(cslice(hm, k), cslice(hm, k),
                                            bmid, None, OP.add)

            # ---- kernel branch: ker = (span@reduce) @ hm  (32-row padded)
            kcf = kcm[:].rearrange("t il w -> t (il w)")
            for k in range(NCHUNK):
                pk = ps.tile([32, CHUNK], F32, tag="mm")
                for m in range(CHUNK // 512):
                    nc.tensor.matmul(pk[:, m * 512:(m + 1) * 512], km[:],
                                     cslice(hm, k)[:, m * 512:(m + 1) * 512],
                                     start=True, stop=True)
                nc.scalar.activation(cslice(kcf, k), pk[:], AT.Copy)

            # j-shifted ker copies in c-major free space (dj = +1 / -1):
            # kcp[t, il, j'] = ker[t, il, j'-1], borders zero
            nc.vector.memset(kcp[:, :, 0:1], 0.0)
            nc.vector.tensor_scalar(kcp[:, :, 1:W], kcm[:, :, 0:W - 1], 1.0,
                                    None, OP.mult)
            nc.vector.memset(kcmm[:, :, W - 1:W], 0.0)
            nc.vector.tensor_scalar(kcmm[:, :, 0:W - 1], kcm[:, :, 1:W], 1.0,
                                    None, OP.mult)

            # ---- transpose h and ker into pixel-major (j; *, il)
            nc.sync.dma_start_transpose(t1[:], hm[:])
            nc.scalar.activation(hT2[:], t1[:].rearrange("j il ch -> j ch il"),
                                 AT.Copy)
            for src, dst in ((kcm, kt), (kcp, ktp), (kcmm, ktm)):
                nc.sync.dma_start_transpose(
                    kt1[:], src[:].rearrange("t il w -> t (il w)"))
                nc.scalar.activation(dst[:],
                                     kt1[:].rearrange("j il t -> j t il"),
                                     AT.Copy)

            # ---- tap loop
            # tap (ti, tj): out[c,i,j] += h[c, i+di, j+dj] * ker[ti*3+tj, i, j]
            # Q_t[j'; ch, il] = hT2[j'; ch, il+di] * ker(t, i, j'-dj)
            # out[j] = sum_t Q_t[j+dj] via shifted-identity matmuls.
            TAPS = [(ti - 1, tj - 1, ti * 3 + tj) for ti in range(3)
                    for tj in range(3)]
            h4 = hT2[:].rearrange("j (h c) il -> j h c il", h=NH)
            for b in range(NBLK):
                o0 = b * BS
                pt = ps.tile([W, CH * BS], F32, tag="mm")
                first = True
                for di, dj, t in TAPS:
                    kv = {1: ktp, 0: kt, -1: ktm}[dj]
                    kv4 = kv[:].rearrange("j (h t) il -> j h t il", h=NH)
                    lhs = {1: ipm, 0: i0, -1: imm}[dj]
                    q = qp.tile([W, CH, BS], BF, tag="q")
                    q4 = q[:].rearrange("j (h c) il -> j h c il", h=NH)
                    r0, r1 = max(o0, -di), min(o0 + BS, IL - di)
                    kb = kv4[:, :, t:t + 1, r0:r1].to_broadcast(
                        [W, NH, C, r1 - r0])
                    nc.vector.tensor_tensor(
                        q4[:, :, :, r0 - o0:r1 - o0],
                        h4[:, :, :, r0 + di:r1 + di], kb, OP.mult)
                    if di == 1 and b == NBLK - 1:
                        # carry: out (h0, il=63) <- in (h1, il=0)
                        kc = kv[:, t:t + 1, IL - 1:IL].to_broadcast([W, C, 1])
                        nc.vector.tensor_tensor(q[:, 0:C, BS - 1:BS],
                                                hT2[:, C:CH, 0:1], kc, OP.mult)
                        nc.vector.memset(q[:, C:CH, BS - 1:BS], 0.0)
                    if di == -1 and b == 0:
                        # carry: out (h1, il=0) <- in (h0, il=63)
                        kc = kv[:, 16 + t:17 + t, 0:1].to_broadcast([W, C, 1])
                        nc.vector.tensor_tensor(q[:, C:CH, 0:1],
                                                hT2[:, 0:C, IL - 1:IL], kc,
                                                OP.mult)
                        nc.vector.memset(q[:, 0:C, 0:1], 0.0)
                    qf = q[:].rearrange("j ch il -> j (ch il)")
                    for m in range(CH * BS // 512):
                        nc.tensor.matmul(pt[:, m * 512:(m + 1) * 512], lhs[:],
                                         qf[:, m * 512:(m + 1) * 512],
                                         start=first, stop=(t == 8))
                    first = False
                # evict restrided: psum (j; ch, il) -> outT (j; il, ch)
                nc.scalar.activation(
                    outT[:, o0:o0 + BS, :].rearrange("j il ch -> j ch il"),
                    pt[:].rearrange("j (ch il) -> j ch il", ch=CH),
                    AT.Copy)

            # ---- back to c-major
            o_cm = h0  # reuse
            nc.sync.dma_start_transpose(
                o_cm[:].rearrange("p (il w) -> p il w", il=IL),
                outT[:].rearrange("j il ch -> j (il ch)"))

            # ---- BN2 stats + apply + relu -> pin
            for k in range(NCHUNK):
                nc.scalar.activation(cslice(scr, k), cslice(o_cm, k), AT.Copy,
                                     accum_out=st[:, k:k + 1])
            for k in range(NCHUNK):
                nc.scalar.activation(cslice(scr, k), cslice(o_cm, k), AT.Square,
                                     accum_out=st[:, 4 + k:5 + k])
            s2, t2v = bn_affine(g2, b2, 4, cc_bufs[1])
            pin = u  # reuse
            for k in range(NCHUNK):
                nc.scalar.activation(cslice(pin, k), cslice(o_cm, k), AT.Relu,
                                     bias=t2v, scale=s2)

            # ---- post conv -> z, BN3 stats
            z = hm  # reuse
            for k in range(NCHUNK):
                pt = ps.tile([CH, CHUNK], F32, tag="mm")
                for m in range(CHUNK // 512):
                    nc.tensor.matmul(pt[:, m * 512:(m + 1) * 512], w3[:],
                                     cslice(pin, k)[:, m * 512:(m + 1) * 512],
                                     start=True, stop=True)
                nc.scalar.activation(cslice(z, k), pt[:], AT.Copy,
                                     accum_out=st[:, k:k + 1])
            for k in range(NCHUNK):
                nc.scalar.activation(cslice(scr, k), cslice(z, k), AT.Square,
                                     accum_out=st[:, 4 + k:5 + k])
            s3, t3v = bn_affine(g3, b3, 8, cc_bufs[2])

            # ---- final: y = (z*s3 + t3) + x, into x_sb, then DMA out
            wtmp = outT[:].rearrange("a b c -> a (b c)")  # bf16 scratch
            for k in range(NCHUNK):
                wk = wtmp[:, k * CHUNK:(k + 1) * CHUNK]
                nc.vector.tensor_scalar(wk, cslice(z, k), s3, t3v,
                                        OP.mult, OP.add)
                nc.vector.tensor_tensor(cslice(xf, k), wk, cslice(xf, k),
                                        OP.add)
                nc.sync.dma_start(
                    out=y_ext[:].rearrange("c (h il) w -> h c il w",
                                           h=NH)[:, :, k * ILC:(k + 1) * ILC, :],
                    in_=x_sb[:, k * ILC:(k + 1) * ILC, :])
    nc.compile()
    return nc


def _prep(inputs):
    f64 = {k: np.asarray(v, np.float64) for k, v in inputs.items()}
    w1 = _bdiag(_bin_w(f64["pre_conv_w"]))
    m = _bin_w(f64["span_w"]) @ _bin_w(f64["reduce_w"])  # (9, 64)
    km = np.zeros((CH, 32), np.float64)
    for h in range(NH):
        km[h * C:(h + 1) * C, h * 16:h * 16 + 9] = m.T
    w3 = _bdiag(_bin_w(f64["post_conv_w"]))
    i0 = np.eye(W)
    ip = np.zeros((W, W)); ip[np.arange(1, W), np.arange(W - 1)] = 1.0
    im = np.zeros((W, W)); im[np.arange(W - 1), np.arange(1, W)] = 1.0
    em = np.zeros((CH, CH), np.float32)
    for h1 in range(NH):
        for h2 in range(NH):
            em[h1 * C + np.arange(C), h2 * C + np.arange(C)] = 1.0

    def chv(v):
        v = np.asarray(v, np.float32).reshape(-1)
        return np.tile(v, NH)

    vecs = np.stack([
        chv(f64["pre_gamma"]), chv(f64["pre_beta"]), chv(f64["pre_a"]),
        chv(f64["mid_gamma"]), chv(f64["mid_beta"]),
        chv(f64["post_gamma"]), chv(f64["post_beta"]),
        chv(f64["mid_bias_b"][0, :, 0, 0]),
    ], axis=1).astype(np.float32)

    bf = ml_dtypes.bfloat16
    return {
        "w1bd": w1.astype(bf), "km32": km.astype(bf), "w3bd": w3.astype(bf),
        "imat0": i0.astype(bf), "imatp": ip.astype(bf), "imatm": im.astype(bf),
        "emat": em.astype(np.float32), "vecs": vecs,
    }, bool(np.any(f64["mid_bias_b"] != 0.0))


def get_nc(with_mid_bias=False):
    key = ("nc", with_mid_bias)
    if key not in _CACHE:
        _CACHE[key] = _build_nc(with_mid_bias)
    return _CACHE[key]


def kernel(**inputs):
    from concourse.bass_utils import run_bass_kernel_spmd

    x = np.ascontiguousarray(np.asarray(inputs["x"], np.float32))
    B = x.shape[0]
    assert B == N_CORES and x.shape[1:] == (C, NH * IL, W)
    weights, with_bias = _prep(inputs)
    nc = get_nc(with_bias)
    in_maps = [dict(weights, x=x[b]) for b in range(B)]
    res = run_bass_kernel_spmd(nc, in_maps, core_ids=list(range(N_CORES)))
    out = np.stack([np.asarray(r["y"], np.float32) for r in res.results])
    return out
